# revision 16
# baseline (speedup 1.0000x reference)
"""Trainium2 Bass kernel for nn_LlamaAttention_45749991637119.

Mathematical structure of the reference: K/V are a single shared head that
is broadcast across all 64 query heads, and attention is computed per token
position (no cross-token mixing).  scores[b,t,h,g] = q[b,t,h]·k[b,t] is
independent of g, so the softmax over g is exactly uniform (1/64) and
attn[b,t,h,:] == v[b,t,:] for every head h.  Therefore

    out = (hidden @ Wv.T) @ Wo_sum.T,   Wo_sum[i,d] = sum_h Wo[i, 64h+d]

and Wq/Wk/cos/sin never influence the output (verified to 5e-7 rel err
against the reference).

Device schedule per core (1024 tokens):

  stage A (v = Wv @ h^T): two 512-token SUPER-groups, col-tiled 2x —
    even k-chunks accumulate into PSUM partitions 0-63 (PE array cols
    0-63), odd chunks into partitions 64-127, CONCURRENTLY.  N=512 moving
    amortizes the ~150ns fixed LDWEIGHTS issue cost (the stage-A pacer).
    Produces a stacked [128, 512] psum: [vE; vO].
  stage B (out = v @ WoSum^T): four 256-token groups; the stacked vT
    (cast to bf16 by ACT) is a K=128 stationary, the moving operand is
    WoSum^T REPLICATED on both partition halves, so the matmul itself
    computes vE·woS + vO·woS = v·woS — full-array K=128 matmuls.
  drain: stage-B PSUM is copied to SBUF in 1024-col PAIRS split across
    Vector (row-block 0) and Scalar (row-block 1) engines — a single
    engine's ~1 elem/lane/cycle PSUM read rate would be the bottleneck.
  stores: one 256KB DMA per drained pair on gpsimd (SWDGE), so output
    DMA trickles out concurrently with everything else.

PE program order: [warmup dummies] A0 B0 B1 A1 B2 B3.  The ~20 dummy
matmuls run during the otherwise-idle DMA lead-in (~7-14us) purely to
hold the PE HAM clock-gate at 8/8 (2.4 GHz) before real work arrives.
ht loads are split in 8-chunk quarters so stage A tracks the DMA.

Sharding: data-parallel over tokens (B*T = 8192 -> 1024 per core).  All
inputs are packed on the host into ONE [128, 38912] bf16 tensor
(Wv^T chunks | WoSum^T x2 | hidden^T super-major).
"""

from contextlib import ExitStack

import numpy as np

import concourse.bass as bass
import concourse.mybir as mybir
from concourse.bass_utils import run_bass_kernel_spmd

N_CORES = 8
B, T, HID = 4, 2048, 4096
D = 64                      # v dim (head_dim)
TOKS = (B * T) // N_CORES   # 1024 tokens per core
P = 128                     # partitions
KC = HID // P               # 32 k-chunks
SG = 512                    # stage-A super-group tokens
NS = TOKS // SG             # 2 supers
TG = 256                    # stage-B token group
NG = TOKS // TG             # 4 groups
CD = 512                    # stage-B out-column tile
NCT = HID // CD             # 8 col tiles
NB = 6                      # stage-B psum ring (3 drain-pairs)
RB = TOKS // P              # 8 row-blocks (2 per group)
N_WARM = 24                 # PE warmup dummy matmuls

# packed input column offsets (bf16 elements per partition)
WV_COLS = KC * D            # 2048
WOS_COLS = HID              # 4096
HT_S_COLS = KC * SG         # 16384 per super
HT0 = WV_COLS + WOS_COLS    # 6144
PACK_COLS = HT0 + NS * HT_S_COLS  # 38912

COMPUTE_DTYPE = "bf16"
_CACHE = {}
LAST_RESULT = None

PE_ORDER = [("A", 0), ("B", 0), ("B", 1), ("A", 1), ("B", 2), ("B", 3)]


def _ticks():
    """Precompute semaphore tick tables for all engines."""
    # s_pe is incremented once per A-group and once per B drain-PAIR
    # (on the pair's second tile) — halving sem-inc traffic on the PE.
    a_tick, b_tick = {}, {}
    pe = 0
    for kind, g in PE_ORDER:
        if kind == "A":
            pe += 1
            a_tick[g] = pe
        else:
            for i in range(16):
                if i % 2 == 1:
                    pe += 1
                    b_tick[(g, i)] = pe

    # Drain-pair -> engine: alternate DVE/ACT at PAIR level within each
    # row-block so both engines drain CONCURRENTLY (rb-level split made
    # them ping-pong, stalling the PE psum ring ~0.5us per pair).
    # ACT additionally does the vT copies, split in 256-col halves so
    # B_g only gates on its own half.
    def on_dve(rb, pi):
        return (pi + rb) % 2 == 0

    dve_prog, act_prog = [], []
    for kind, g in PE_ORDER:
        if kind == "A":
            act_prog.append(("vt", g, 0))
            act_prog.append(("vt", g, 1))
        else:
            for rb in range(2):
                for pi in range(4):
                    (dve_prog if on_dve(rb, pi) else act_prog).append(
                        ("dr", g, rb, pi))

    vt_tick, pair_tick, pair_on_dve = {}, {}, {}
    t = 0
    for op in dve_prog:
        t += 1
        pair_tick[op[1:]] = t
        pair_on_dve[op[1:]] = True
    t = 0
    for op in act_prog:
        t += 1
        if op[0] == "vt":
            vt_tick[(op[1], op[2])] = t
        else:
            pair_tick[op[1:]] = t
            pair_on_dve[op[1:]] = False
    return a_tick, b_tick, dve_prog, act_prog, vt_tick, pair_tick, pair_on_dve


def _build():
    dt_in = mybir.dt.bfloat16

    nc = bass.Bass()
    pack = nc.dram_tensor("pack", [P, PACK_COLS], dt_in, kind="ExternalInput")
    out = nc.dram_tensor("out", [TOKS, HID], dt_in, kind="ExternalOutput")

    (a_tick, b_tick, dve_prog, act_prog, vt_tick, pair_tick,
     pair_on_dve) = _ticks()

    # Load gating uses ONE SEMAPHORE PER WAIT-GROUP, each waited at its
    # FINAL value.  A single cumulative load-semaphore is UNSOUND: the 16
    # SDMA engines increment independently, so an intermediate threshold
    # like ">=32" can be reached while an early piece is still incomplete
    # on a lagging engine (observed as NaN outputs).  A sem's final value
    # is exact, and per-engine FIFO gives prefix-closure across pieces.
    # groups: e0s(wv+e0)=32, e1s..e6s=16, we7(woS2a+e7)=32, wbs(woS2b)=16,
    # q0s..q3s=16

    with ExitStack() as ctx:
        mega = ctx.enter_context(nc.sbuf_tensor("mega", [P, PACK_COLS], dt_in))
        out_sb = ctx.enter_context(nc.sbuf_tensor("out_sb", [P, RB * HID], dt_in))
        vT = ctx.enter_context(nc.sbuf_tensor("vT", [P, TOKS], dt_in))
        psv0 = ctx.enter_context(nc.psum_tensor("psv0", [P, SG]))
        psv1 = ctx.enter_context(nc.psum_tensor("psv1", [P, SG]))
        psB = ctx.enter_context(nc.psum_tensor("psB", [P, NB * CD]))
        s_e = [ctx.enter_context(nc.semaphore(name=f"e{i}s")) for i in range(7)]
        swe7 = ctx.enter_context(nc.semaphore(name="we7"))
        s_e.append(swe7)
        swb = ctx.enter_context(nc.semaphore(name="wbs"))
        s_q = [ctx.enter_context(nc.semaphore(name=f"q{i}s")) for i in range(4)]
        s_pe = ctx.enter_context(nc.semaphore(name="s_pe"))
        s_dve = ctx.enter_context(nc.semaphore(name="s_dve"))
        s_act = ctx.enter_context(nc.semaphore(name="s_act"))
        s_store = ctx.enter_context(nc.semaphore(name="s_store"))
        block = ctx.enter_context(nc.Block())

        psv = [psv0, psv1]
        # (sem, final value) for each stage-A0 eighth
        e_gate = [(s_e[0], 32)] + [(s_e[i], 16) for i in range(1, 7)] + [(swe7, 32)]

        def wv_chunk(c):
            return mega[:, c * D:(c + 1) * D]

        def woS2(ct):
            return mega[:, WV_COLS + ct * CD:WV_COLS + (ct + 1) * CD]

        def ht(s, c):
            base = HT0 + s * HT_S_COLS + c * SG
            return mega[:, base:base + SG]

        @block.sync
        def _(sync):
            q = HT_S_COLS // 4   # 4096 cols = 8 chunks
            e = HT_S_COLS // 8   # 2048 cols = 4 chunks
            s0, s1 = HT0, HT0 + HT_S_COLS
            wmid = WV_COLS + WOS_COLS // 2
            pieces = [((0, WV_COLS), s_e[0])]
            pieces += [((s0 + i * e, s0 + (i + 1) * e), s_e[i]) for i in range(7)]
            pieces += [((WV_COLS, wmid), swe7), ((s0 + 7 * e, s1), swe7),
                       ((wmid, HT0), swb)]
            pieces += [((s1 + i * q, s1 + (i + 1) * q), s_q[i]) for i in range(4)]
            for (lo, hi), sem in pieces:
                sync.dma_start(out=mega[:, lo:hi], in_=pack[:, lo:hi]).then_inc(
                    sem, 16
                )
            # stores: HWDGE on the (otherwise idle) sync engine — avoids
            # the multi-us SWDGE ring-drain postamble gpsimd stores pay.
            # One 256KB store per drained pair, single wait each.
            n_store = 0
            for _, g in [x for x in PE_ORDER if x[0] == "B"]:
                for rb in range(2):
                    r = g * 2 + rb
                    for pi in range(4):
                        key = (g, rb, pi)
                        if pair_on_dve[key]:
                            sync.wait_ge(s_dve, pair_tick[key])
                        else:
                            sync.wait_ge(s_act, pair_tick[key])
                        c0 = 2 * pi * CD
                        sync.dma_start(
                            out=out[r * P:(r + 1) * P, c0:c0 + 2 * CD],
                            in_=out_sb[:, r * HID + c0:r * HID + c0 + 2 * CD],
                        ).then_inc(s_store, 16)
                        n_store += 1
            sync.wait_ge(s_store, 16 * n_store)

        @block.tensor
        def _(tensor):
            waited = {}

            def wait(sem, name, val):
                if waited.get(name, 0) < val:
                    waited[name] = val
                    tensor.wait_ge(sem, val)

            # Warmup: keep the PE busy during the DMA lead-in so HAM
            # un-throttles to 2.4 GHz before real matmuls arrive.  Reads
            # uninitialized SBUF (harmless); results overwritten by the
            # first real start=True matmul into each psB slot.
            for w in range(N_WARM):
                tensor.matmul(
                    psB[:, (w % NB) * CD:(w % NB) * CD + TG],
                    mega[:, 0:P],
                    mega[:, 0:TG],
                    start=True, stop=True,
                )

            def mini_warm(n=2):
                # tiny dummy matmuls emitted just before a wait that may
                # stall on DMA: keeps the PE HAM activity window busy so
                # the clock stays at 8/8 through stage-A's paced stalls.
                for w in range(n):
                    tensor.matmul(
                        psB[:, 0:P], mega[:, 0:P], mega[:, 0:P],
                        start=True, stop=True, skip_group_check=True,
                    )

            # Emission plan: A1's chunk-pairs are INTERLEAVED into B0's
            # tail and B1 so they execute inside the stage-B drain-ring
            # stalls instead of occupying a dedicated serial PE phase.
            # Interleaved A matmuls don't increment s_pe, so all tick
            # values are identical to the serial A0,B0,B1,A1,B2,B3 order.
            plan = [("A", 0, c) for c in range(KC)]
            for i in range(16):
                plan.append(("B", 0, i))
                if i >= 8 and i % 2 == 0:
                    p = (i - 8) // 2
                    plan += [("A", 1, 2 * p), ("A", 1, 2 * p + 1)]
            for i in range(16):
                plan.append(("B", 1, i))
                if i % 2 == 1:
                    p = 4 + (i - 1) // 2
                    plan += [("A", 1, 2 * p), ("A", 1, 2 * p + 1)]
            plan += [("A", 1, c) for c in range(24, KC)]
            plan += [("B", 2, i) for i in range(16)]
            plan += [("B", 3, i) for i in range(16)]

            b_started = set()
            for kind, g, x in plan:
                if kind == "A":
                    c = x
                    if g == 0 and c % 4 == 0:
                        mini_warm(6 if c == 28 else 2)
                        sem, val = e_gate[c // 4]
                        wait(sem, f"e{c // 4}", val)
                    elif g == 1 and c % 8 == 0:
                        wait(s_q[c // 8], f"q{c // 8}", 16)
                    half = c % 2
                    mm = tensor.matmul(
                        psv[g][half * D:(half + 1) * D, :],
                        wv_chunk(c),
                        ht(g, c),
                        start=(c < 2),
                        stop=(c >= KC - 2),
                        skip_group_check=True,
                    )
                    if c == KC - 1:
                        mm.then_inc(s_pe, 1)
                else:
                    i = x
                    if g not in b_started:
                        b_started.add(g)
                        if g == 0:
                            mini_warm(3)
                        wait(s_act, "act", vt_tick[(g // 2, g % 2)])
                        if g == 0:
                            wait(swe7, "e7", 32)
                    if g == 0 and i == 4:
                        wait(swb, "wb", 16)
                    j = g * 16 + i
                    if j >= NB and j % 2 == 0:
                        # one WAR wait covers both tiles of the incoming
                        # pair: the drain-pair of (j-6, j-5) frees both
                        # ring slots at once.
                        gp, ip = divmod(j - NB, 16)
                        key = (gp, ip // 8, (ip % 8) // 2)
                        if pair_on_dve[key]:
                            wait(s_dve, "dve", pair_tick[key])
                        else:
                            wait(s_act, "act", pair_tick[key])
                    slot = j % NB
                    rb, ct = divmod(i, 8)
                    mm = tensor.matmul(
                        psB[:, slot * CD:(slot + 1) * CD],
                        vT[:, (g * 2 + rb) * P:(g * 2 + rb + 1) * P],
                        woS2(ct),
                        start=True, stop=True,
                        skip_group_check=True,
                    )
                    if i % 2 == 1:
                        mm.then_inc(s_pe, 1)

        @block.vector
        def _(vector):
            for _, g, rb, pi in dve_prog:
                i = rb * 8 + 2 * pi
                j = g * 16 + i
                vector.wait_ge(s_pe, b_tick[(g, i + 1)])
                slot = j % NB
                r = g * 2 + rb
                vector.tensor_copy(
                    out=out_sb[:, r * HID + 2 * pi * CD:r * HID + (2 * pi + 2) * CD],
                    in_=psB[:, slot * CD:(slot + 2) * CD],
                ).then_inc(s_dve, 1)

        @block.scalar
        def _(scalar):
            # preload the activation table set (Copy) during the DMA
            # lead-in so the one-time ~1.5us ACT_TABLE_LOAD is off the
            # critical path; reads uninitialized psum, result unused.
            scalar.activation(
                out=vT[0:1, 0:8], in_=psv0[0:1, 0:8],
                func=mybir.ActivationFunctionType.Copy,
            )
            for op in act_prog:
                if op[0] == "vt":
                    _, s, h = op
                    hw = SG // 2
                    scalar.wait_ge(s_pe, a_tick[s])
                    scalar.activation(
                        out=vT[:, s * SG + h * hw:s * SG + (h + 1) * hw],
                        in_=psv[s][:, h * hw:(h + 1) * hw],
                        func=mybir.ActivationFunctionType.Copy,
                    ).then_inc(s_act, 1)
                else:
                    _, g, rb, pi = op
                    i = rb * 8 + 2 * pi
                    j = g * 16 + i
                    scalar.wait_ge(s_pe, b_tick[(g, i + 1)])
                    slot = j % NB
                    r = g * 2 + rb
                    scalar.activation(
                        out=out_sb[:, r * HID + 2 * pi * CD:
                                   r * HID + (2 * pi + 2) * CD],
                        in_=psB[:, slot * CD:(slot + 2) * CD],
                        func=mybir.ActivationFunctionType.Copy,
                    ).then_inc(s_act, 1)

    return nc


def kernel(hidden_states, cos, sin, Wq, Wk, Wv, Wo):
    global LAST_RESULT
    import ml_dtypes
    np_bf16 = ml_dtypes.bfloat16

    if "nc" not in _CACHE:
        _CACHE["nc"] = _build()
    nc = _CACHE["nc"]

    hidden_states = np.asarray(hidden_states, dtype=np.float32)
    Wv = np.asarray(Wv, dtype=np.float32)
    Wo = np.asarray(Wo, dtype=np.float32)

    flat = hidden_states.reshape(B * T, HID)
    # Wv^T chunks: pack[p, c*64+d] = Wv[d, c*128+p]
    wv_part = np.ascontiguousarray(
        Wv.reshape(D, KC, P).transpose(2, 1, 0).reshape(P, KC * D)
    ).astype(np_bf16)
    # Wo_sum^T replicated on both partition halves: pack[p, j] = woS[p%64, j]
    woS = Wo.reshape(HID, HID // D, D).sum(axis=1, dtype=np.float32).T  # [64, 4096]
    woS2_part = np.concatenate([woS, woS], axis=0).astype(np_bf16)      # [128, 4096]

    in_maps = []
    for jc in range(N_CORES):
        blk = flat[jc * TOKS:(jc + 1) * TOKS, :]          # [1024, 4096]
        # ht super-major: pack[p, s*16384 + c*512 + t] = blk[s*512+t, c*128+p]
        ht_part = np.ascontiguousarray(
            blk.reshape(NS, SG, KC, P).transpose(3, 0, 2, 1).reshape(P, NS * HT_S_COLS)
        ).astype(np_bf16)
        packed = np.concatenate([wv_part, woS2_part, ht_part], axis=1)
        in_maps.append({"pack": np.ascontiguousarray(packed)})

    LAST_RESULT = run_bass_kernel_spmd(nc, in_maps, core_ids=list(range(N_CORES)))
    outs = [np.asarray(LAST_RESULT.results[jc]["out"]).astype(np.float32)
            for jc in range(N_CORES)]
    return np.concatenate(outs, axis=0).reshape(B, T, HID)
